# revision 1
# baseline (speedup 1.0000x reference)
"""Trainium2 Bass kernel for the Darcy64 residual (dense stencil + BC extraction).

Contract: kernel(**inputs) takes the FULL inputs from setup_inputs()
(x0_pred [2048,2,64,64] f32, compute_bc scalar) and returns the FULL
output [2048,3,64,64] f32 (or [2048,1,64,64] if compute_bc is falsy).

Strategy: pure data parallel over 8 NeuronCores (256 samples each).
Per core, samples sit on SBUF partitions (128 per tile, 2 tiles) and each
sample's [2,64,64] grid is flattened along the free dim.  All stencils are
free-dim shifted scalar_tensor_tensor / tensor_tensor ops, split across the
Vector (DVE) and GpSimd engines, with the Scalar (ACT) engine handling
boundary-condition extraction and source-term corners.

Math (d = 1/64, flat index = i*64 + j):
  a = 23*(x0+1), p = 1.7*x1
  res0 = -a*(p_xx + p_yy) - a_x*p_x - a_y*p_y - f_s
       = -C*(x0+1)*S2 - C4*(A0*P0 + A1*P1) - f_s
  with C = 39.1/d^2, C4 = C/4, S2 = Dxx(x1)+Dyy(x1), and P/A raw
  (unscaled) central differences with one-sided 2nd-order ends.
  First-derivative end rows/cols are computed with the *first*-end
  coefficients at both ends, which flips the sign of the last row/col;
  the flip cancels in the products A*P and makes the BC scale uniform.
  f_s is zero except +10 on grid [0:8,0:8] and -10 on [56:64,56:64].
  BC: out[:,1,{0,63},:] = -54.4*P0 rows; out[:,2,:,{0,63}] = +54.4*P1 cols.
  All other BC-plane entries are zero; ch1 relies on the runtime's
  pre-zeroed output buffers and only the two rows per sample are stored.
"""

import sys
from contextlib import ExitStack

import numpy as np

sys.path.insert(0, "/opt/trn_rl_repo")

import concourse.bass as bass  # noqa: E402
import concourse.tile as tile  # noqa: E402
from concourse import mybir  # noqa: E402

N_CORES = 8
B = 2048
S_PER_CORE = B // N_CORES  # 256
P = 128                    # samples per tile (partition dim)
N = 64
G = N * N                  # 4096
C = 39.1 * float(N * N)    # 39.1 / d^2 = 160153.6
C4 = C / 4.0
BC_SCALE = 1.7 * (N / 2.0)  # 1.7/(2d) = 54.4

F32 = mybir.dt.float32
ALU = mybir.AluOpType
COPY = mybir.ActivationFunctionType.Copy


def _emit_tile(tc, x_ap, out_ap, s0, dve_slots, sc_shared, gp_slots, xp,
               xts, bc1_pool, bc2, last_tile):
    """Emit one 128-sample tile starting at sample s0 (within this core).

    dve_slots = (Sa, Sb) private to this tile; sc_shared = Sc shared across
    tiles; gp_slots = (Sd, Se) shared GpSimd chain slots; v_pool holds V'.
    """
    nc = tc.nc
    sa, sb = dve_slots
    sc = sc_shared
    sd, se = gp_slots

    x1t, x0t = xts
    x0 = x0t[:]
    x1 = x1t[:]
    x0v = x0.rearrange("p (h w) -> p h w", h=N)
    x1v = x1.rearrange("p (h w) -> p h w", h=N)

    def rows(ap3, k):
        return ap3[:, k:N - k:(N - 1 - 2 * k), :]

    def cols(ap3, k):
        return ap3[:, :, k:N - k:(N - 1 - 2 * k)]

    def _end_views(ap3, k, axis):
        # single-end views: axis 0 -> rows k and N-1-k, axis 1 -> cols
        if axis == 0:
            return ap3[:, k:k + 1, :], ap3[:, N - 1 - k:N - k, :]
        return ap3[:, :, k:k + 1], ap3[:, :, N - 1 - k:N - k]

    def fix_first(dst, src, view):
        # per-end contiguous/1-D ops: the fused 2-block strided APs are
        # pathologically slow under concurrent DMA traffic
        axis = 0 if view is rows else 1
        for e in range(2):
            d = _end_views(dst, 0, axis)[e]
            s0 = _end_views(src, 0, axis)[e]
            s1 = _end_views(src, 1, axis)[e]
            s2 = _end_views(src, 2, axis)[e]
            nc.vector.scalar_tensor_tensor(d, s0, -3.0, s2,
                                           ALU.mult, ALU.subtract)
            nc.vector.scalar_tensor_tensor(d, s1, 4.0, d,
                                           ALU.mult, ALU.add)

    def fix_second(dst, src, view):
        axis = 0 if view is rows else 1
        for e in range(2):
            d = _end_views(dst, 0, axis)[e]
            s0 = _end_views(src, 0, axis)[e]
            s1 = _end_views(src, 1, axis)[e]
            s2 = _end_views(src, 2, axis)[e]
            s3 = _end_views(src, 3, axis)[e]
            nc.vector.scalar_tensor_tensor(d, s0, 2.0, s3,
                                           ALU.mult, ALU.subtract)
            nc.vector.scalar_tensor_tensor(d, s1, -5.0, d,
                                           ALU.mult, ALU.add)
            nc.vector.scalar_tensor_tensor(d, s2, 4.0, d,
                                           ALU.mult, ALU.add)

    sav = sa.rearrange("p (h w) -> p h w", h=N)
    sbv = sb.rearrange("p (h w) -> p h w", h=N)
    scv = sc.rearrange("p (h w) -> p h w", h=N)
    sdv = sd.rearrange("p (h w) -> p h w", h=N)
    sev = se.rearrange("p (h w) -> p h w", h=N)

    # ================= GpSimd chain (independent of DVE) =================
    # P1 = Dy(x1) -> Sd ; A1 = Dy(x0) -> Se ; V' = A1*P1 -> v_pool
    nc.gpsimd.tensor_sub(sd[:, 1:G - 1], x1[:, 2:G], x1[:, 0:G - 2])
    fix_first(sdv, x1v, cols)
    bc2v = bc2.rearrange("p (h w) -> p h w", h=N)
    nc.scalar.activation(cols(bc2v, 0), cols(sdv, 0), COPY,
                         bias=0.0, scale=BC_SCALE)
    nc.scalar.dma_start(
        out=out_ap[s0:s0 + P, 2].rearrange("s h w -> s (h w)"), in_=bc2[:]
    )
    nc.gpsimd.tensor_sub(se[:, 1:G - 1], x0[:, 2:G], x0[:, 0:G - 2])
    fix_first(sev, x0v, cols)

    # ================= DVE chain =================
    # P0 = Dx(x1) -> Sa
    nc.vector.tensor_sub(sa[:, N:G - N], x1[:, 2 * N:G], x1[:, 0:G - 2 * N])
    fix_first(sav, x1v, rows)
    bc1 = bc1_pool.tile([P, 2, N], F32, tag="bc1")
    nc.scalar.activation(bc1[:], rows(sav, 0), COPY, bias=0.0, scale=-BC_SCALE)
    nc.scalar.dma_start(out=out_ap[s0:s0 + P, 1, 0:N:N - 1, :], in_=bc1[:])
    # A0 = Dx(x0) -> Sb ; U = C4*A0*P0 -> Sb (in place)
    nc.vector.tensor_sub(sb[:, N:G - N], x0[:, 2 * N:G], x0[:, 0:G - 2 * N])
    fix_first(sbv, x0v, rows)
    nc.vector.scalar_tensor_tensor(sb, sb, C4, sa, ALU.mult, ALU.mult)
    # V' = A1*P1 in place over P1 (DVE: GpSimd product ops degrade badly
    # under concurrent DMA SBUF traffic)
    nc.vector.tensor_mul(sd, se, sd)
    # x0p1 = x0 + 1 reuses Se once V' has consumed A1
    nc.scalar.add(se, x0, 1.0)
    # Laplacian: t0 -> Sa (over P0), Q0 -> Sa ip, t1 -> Sc, Q1 -> Sc ip
    nc.vector.tensor_add(sa[:, N:G - N], x1[:, 2 * N:G], x1[:, 0:G - 2 * N])
    nc.vector.scalar_tensor_tensor(sa[:, N:G - N], x1[:, N:G - N], -2.0,
                                   sa[:, N:G - N], ALU.mult, ALU.add)
    fix_second(sav, x1v, rows)
    nc.vector.tensor_add(sc[:, 1:G - 1], x1[:, 2:G], x1[:, 0:G - 2])
    nc.vector.scalar_tensor_tensor(sc[:, 1:G - 1], x1[:, 1:G - 1], -2.0,
                                   sc[:, 1:G - 1], ALU.mult, ALU.add)
    fix_second(scv, x1v, cols)
    # S2 = Q0 + Q1 -> Sa, T' = x0p1*S2 -> Sa (GpSimd; DVE on the last
    # tile to shorten the exposed tail)
    tail = nc.vector if last_tile else nc.gpsimd
    tail.tensor_add(sa, sa, sc)
    tail.tensor_mul(sa, se, sa)
    # r1 = -C*T' - U -> Sa ; res = -C4*V' + r1 -> Sa
    nc.vector.scalar_tensor_tensor(sa, sa, -C, sb, ALU.mult, ALU.subtract)
    nc.vector.scalar_tensor_tensor(sa, sd, -C4, sa, ALU.mult, ALU.add)
    # source-term corners: res[0:8,0:8] -= 10 ; res[56:64,56:64] += 10
    nc.scalar.activation(sav[:, 0:8, 0:8], sav[:, 0:8, 0:8], COPY,
                         bias=-10.0, scale=1.0)
    nc.scalar.activation(sav[:, N - 8:N, N - 8:N], sav[:, N - 8:N, N - 8:N],
                         COPY, bias=10.0, scale=1.0)
    nc.scalar.dma_start(
        out=out_ap[s0:s0 + P, 0].rearrange("s h w -> s (h w)"), in_=sa
    )


_WAITSPLIT_N = [0]


def _split_excess_waits(nc, max_waits=1):
    """Engine compute-instruction ISA structs hold only one sync-wait slot;
    Tile can assign several at cross-engine join points ("Too many sync wait
    commands" at codegen).  Move all but one wait onto InstNoOp carriers
    inserted just before, on the same engine."""
    keep = (mybir.InstEventSemaphore,
            mybir.InstCall, mybir.InstUnconditionalBranch, mybir.InstNoOp,
            mybir.InstRegisterMove, mybir.InstISA)
    for f in nc.m.functions:
        for b in f.blocks:
            new_insts = []
            for inst in b.instructions:
                si = inst.sync_info
                if (si is not None and si.on_wait and len(si.on_wait) > max_waits
                        and not isinstance(inst, keep)
                        and getattr(inst, "engine", None) is not None):
                    waits = list(si.on_wait)
                    excess, rest = waits[:-max_waits], waits[-max_waits:]
                    for w in excess:
                        _WAITSPLIT_N[0] += 1
                        nop = mybir.InstNoOp(
                            name=f"waitsplit_{_WAITSPLIT_N[0]}",
                            engine=inst.engine,
                            sync_info=mybir.SyncInfo(on_wait=[w], on_update=[]),
                            bass_nofuse=True,
                        )
                        new_insts.append(nop)
                    inst.sync_info = mybir.SyncInfo(on_wait=rest,
                                                    on_update=list(si.on_update))
                new_insts.append(inst)
            b.instructions = new_insts


def build_bass(split_waits=True):
    nc = bass.Bass()
    x = nc.declare_dram_parameter("x", [S_PER_CORE, 2, N, N], F32,
                                  isOutput=False)
    out = nc.declare_dram_parameter("out", [S_PER_CORE, 3, N, N], F32,
                                    isOutput=True)
    with tile.TileContext(nc) as tc:
        with ExitStack() as ctx:
            bc1_pool = ctx.enter_context(tc.tile_pool(name="bc1", bufs=1))
            sc_pool = ctx.enter_context(tc.tile_pool(name="scratch", bufs=1))
            # two private DVE slot sets + one shared third slot
            sets = []
            for t in range(2):
                sa = sc_pool.tile([P, G], F32, tag=f"sa{t}", name=f"sa{t}")
                sb = sc_pool.tile([P, G], F32, tag=f"sb{t}", name=f"sb{t}")
                sets.append((sa, sb))
            sc_shared = sc_pool.tile([P, G], F32, tag="scs", name="scs")
            sd0 = sc_pool.tile([P, G], F32, tag="sd0", name="sd0")
            sd1 = sc_pool.tile([P, G], F32, tag="sd1", name="sd1")
            se = sc_pool.tile([P, G], F32, tag="se", name="se")
            # persistent dense zero plane for BC channel 2 (memset once;
            # only cols {0,63} are rewritten each tile)
            bc2 = sc_pool.tile([P, G], F32, tag="bc2", name="bc2")
            nc.gpsimd.memset(bc2[:], 0.0)
            n_tiles = S_PER_CORE // P
            # dedicated per-tile x1 slots (both loads start immediately) and
            # one shared x0 slot (its readers all finish early in a tile)
            x1s = [sc_pool.tile([P, G], F32, tag=f"x1s{t}", name=f"x1s{t}")
                   for t in range(n_tiles)]
            x0s = sc_pool.tile([P, G], F32, tag="x0s", name="x0s")

            def load(it, ch, dst):
                nc.sync.dma_start(
                    out=dst[:],
                    in_=x[:][it * P:(it + 1) * P, ch].rearrange(
                        "s h w -> s (h w)"))
                return dst

            for it in range(n_tiles):
                load(it, 1, x1s[it])
            for it in range(n_tiles):
                load(it, 0, x0s)
                _emit_tile(tc, x[:], out[:], it * P, sets[it % 2], sc_shared,
                           ([sd0, sd1][it % 2], se[:]), None,
                           (x1s[it], x0s),
                           bc1_pool, bc2, last_tile=(it == n_tiles - 1))
    if split_waits:
        _split_excess_waits(nc)
    return nc


_NC = None


def _get_nc():
    global _NC
    if _NC is None:
        _NC = build_bass()
    return _NC


def _axon_device_reset():
    """Recover a wedged NeuronCore (NRT_EXEC_UNIT_UNRECOVERABLE) via the
    axon client's reset entry point."""
    try:
        import ctypes

        import jax

        jax.devices()
        lib = ctypes.CDLL("/opt/axon/libaxon_pjrt.so")
        lib.axon_reset.restype = ctypes.c_int64
        return int(lib.axon_reset()) == 0
    except Exception:
        return False


def kernel(x0_pred, compute_bc=1, **_):
    from concourse.bass_utils import run_bass_kernel_spmd

    x = np.ascontiguousarray(np.asarray(x0_pred), dtype=np.float32)
    assert x.shape == (B, 2, N, N), x.shape
    nc = _get_nc()
    shards = x.reshape(N_CORES, S_PER_CORE, 2, N, N)
    in_maps = [{"x": shards[i]} for i in range(N_CORES)]
    try:
        res = run_bass_kernel_spmd(nc, in_maps, list(range(N_CORES)))
    except Exception:
        if not _axon_device_reset():
            raise
        res = run_bass_kernel_spmd(nc, in_maps, list(range(N_CORES)))
    full = np.concatenate([res.results[i]["out"] for i in range(N_CORES)],
                          axis=0)
    if not int(np.asarray(compute_bc)):
        return full[:, :1]
    return full



# revision 6
# speedup vs baseline: 1.1309x; 1.1309x over previous
"""Trainium2 Bass kernel for the Darcy64 residual (dense stencil + BC extraction).

Contract: kernel(**inputs) takes the FULL inputs from setup_inputs()
(x0_pred [2048,2,64,64] f32, compute_bc scalar) and returns the FULL
output [2048,3,64,64] f32 (or [2048,1,64,64] if compute_bc is falsy).

Strategy: pure data parallel over 8 NeuronCores (256 samples each),
128 samples per tile on SBUF partitions, each sample's [2,64,64] grid
flattened along the free dim.  v2 changes vs the fp32 baseline:

  * All compute in bf16 -> DVE tensor_tensor/scalar_tensor_tensor run in
    the 2x_1p perf mode (two elements/cycle) instead of fp32 1x.
  * Both channels stacked in one [128, 8192] tile (xc = [x0 | x1]) so the
    i-direction stencils for both channels are single wide ops; the
    cross-channel seam garbage lands on end rows that boundary fixes
    rewrite anyway.
  * j-direction stencils write into +1-shifted buffers (buf[k] = val[k-1])
    so every big DVE op keeps 4-byte-aligned APs (2x mode requires it).
    The two parity-crossing merges (q1 = -2*x1 + t_j, V = A1*P1) run on
    GpSimd, which has no alignment-gated perf modes.
  * Inputs are cast fp32->bf16 during the DMA load (SWDGE); the residual
    and the dense-zero BC ch2 plane are cast bf16->fp32 during the store.

Math (d = 1/64, flat index = i*64 + j):
  a = 23*(x0+1), p = 1.7*x1
  res0 = -C*(x0+1)*S2 - C4*(A0*P0 + A1*P1) - f_s
  with C = 39.1/d^2, C4 = C/4, S2 = Dxx(x1)+Dyy(x1), and P/A raw
  (unscaled) central differences with one-sided 2nd-order ends.
  First-derivative end rows/cols use the *first*-end coefficients at both
  ends, which flips the sign of the last row/col; the flip cancels in the
  products A*P and makes the BC scale uniform.
  f_s is zero except +10 on grid [0:8,0:8] and -10 on [56:64,56:64].
  BC: out[:,1,{0,63},:] = -54.4*P0 rows; out[:,2,:,{0,63}] = +54.4*P1 cols.
  All other BC-plane entries are zero; ch1 relies on the runtime's
  pre-zeroed output buffers and only the two rows per sample are stored.
"""

import sys
from contextlib import ExitStack

import numpy as np

sys.path.insert(0, "/opt/trn_rl_repo")

import concourse.bass as bass  # noqa: E402
import concourse.tile as tile  # noqa: E402
from concourse import mybir  # noqa: E402

N_CORES = 8
B = 2048
S_PER_CORE = B // N_CORES  # 256
P = 128                    # samples per tile (partition dim)
N = 64
G = N * N                  # 4096
G2 = 2 * G                 # 8192 (two stacked channels)
C = 39.1 * float(N * N)    # 39.1 / d^2 = 160153.6
C4 = C / 4.0
BC_SCALE = 1.7 * (N / 2.0)  # 1.7/(2d) = 54.4

F32 = mybir.dt.float32
BF16 = mybir.dt.bfloat16
ALU = mybir.AluOpType
COPY = mybir.ActivationFunctionType.Copy


def _fix_first_rows(nc, dstv, srcv, ch):
    """First-derivative one-sided ends on rows 0 and 63 of channel block
    `ch` of a stacked [p, 2, 64, 64] pair (dstv), sources from srcv.
    Uses first-end coefficients mirrored at the far end (sign flip there
    cancels in products / makes BC scale uniform)."""
    for (r0, r1, r2) in ((0, 1, 2), (N - 1, N - 2, N - 3)):
        d = dstv[:, ch, r0:r0 + 1, :]
        f0 = srcv[:, ch, r0:r0 + 1, :]
        f1 = srcv[:, ch, r1:r1 + 1, :]
        f2 = srcv[:, ch, r2:r2 + 1, :]
        nc.vector.scalar_tensor_tensor(d, f0, -3.0, f2, ALU.mult, ALU.subtract)
        nc.vector.scalar_tensor_tensor(d, f1, 4.0, d, ALU.mult, ALU.add)


def _fix_second_rows(nc, dstv, srcv):
    """Second-derivative one-sided ends on rows 0 and 63 (single-channel
    [p, 64, 64] views)."""
    for (r0, r1, r2, r3) in ((0, 1, 2, 3), (N - 1, N - 2, N - 3, N - 4)):
        d = dstv[:, r0:r0 + 1, :]
        f0 = srcv[:, r0:r0 + 1, :]
        f1 = srcv[:, r1:r1 + 1, :]
        f2 = srcv[:, r2:r2 + 1, :]
        f3 = srcv[:, r3:r3 + 1, :]
        nc.vector.scalar_tensor_tensor(d, f0, 2.0, f3, ALU.mult, ALU.subtract)
        nc.vector.scalar_tensor_tensor(d, f1, -5.0, d, ALU.mult, ALU.add)
        nc.vector.scalar_tensor_tensor(d, f2, 4.0, d, ALU.mult, ALU.add)


def _fix_first_cols(nc, dstv, srcv):
    """First-derivative one-sided ends on cols 0 and 63 (single-channel
    [p, 64, 64] views; strided APs)."""
    for (c0, c1, c2) in ((0, 1, 2), (N - 1, N - 2, N - 3)):
        d = dstv[:, :, c0:c0 + 1]
        f0 = srcv[:, :, c0:c0 + 1]
        f1 = srcv[:, :, c1:c1 + 1]
        f2 = srcv[:, :, c2:c2 + 1]
        nc.vector.scalar_tensor_tensor(d, f0, -3.0, f2, ALU.mult, ALU.subtract)
        nc.vector.scalar_tensor_tensor(d, f1, 4.0, d, ALU.mult, ALU.add)


def _fix_second_cols(nc, dstv, srcv):
    for (c0, c1, c2, c3) in ((0, 1, 2, 3), (N - 1, N - 2, N - 3, N - 4)):
        d = dstv[:, :, c0:c0 + 1]
        f0 = srcv[:, :, c0:c0 + 1]
        f1 = srcv[:, :, c1:c1 + 1]
        f2 = srcv[:, :, c2:c2 + 1]
        f3 = srcv[:, :, c3:c3 + 1]
        nc.vector.scalar_tensor_tensor(d, f0, 2.0, f3, ALU.mult, ALU.subtract)
        nc.vector.scalar_tensor_tensor(d, f1, -5.0, d, ALU.mult, ALU.add)
        nc.vector.scalar_tensor_tensor(d, f2, 4.0, d, ALU.mult, ALU.add)


def _emit_tile(tc, x_ap, out_ap, s0, bufs, bc2, x1d, last_tile):
    """Emit one 128-sample tile starting at sample s0 (within this core)."""
    nc = tc.nc
    xc, pai, tiq, q0, u, pj0, pj1, tjq, q1, v, bc1 = bufs

    # stacked [x0 | x1] channel views
    x0 = xc[:, 0:G]
    x1 = xc[:, G:G2]
    xcv = xc[:].rearrange("p (c h w) -> p c h w", c=2, h=N)
    x0v = xcv[:, 0]
    x1v = xcv[:, 1]

    # ---- load: fp32 DRAM -> bf16 SBUF (SWDGE cast) -----------------------
    nc.gpsimd.dma_start(
        out=xc[:],
        in_=x_ap[s0:s0 + P].rearrange("s c h w -> s (c h w)"),
    )

    # ---- j-direction stencils into +1-shifted buffers (all APs even) -----
    # tjq[k] = x1[k-1+1] + x1[k-1-1] = x1[k] + x1[k-2]  (t_j shifted +1)
    nc.vector.tensor_add(tjq[:, 2:G], x1[:, 2:G], x1[:, 0:G - 2])
    # pj0[k] = A1c[k-1] = x0[k] - x0[k-2], pj1[k] = P1c[k-1]
    nc.vector.tensor_sub(pj0[:, 2:G], x0[:, 2:G], x0[:, 0:G - 2])
    nc.vector.tensor_sub(pj1[:, 2:G], x1[:, 2:G], x1[:, 0:G - 2])

    # natural-position views of the shifted buffers
    pj0n = pj0[:, 1:1 + G].rearrange("p (h w) -> p h w", h=N)
    pj1n = pj1[:, 1:1 + G].rearrange("p (h w) -> p h w", h=N)
    _fix_first_cols(nc, pj0n, x0v)
    _fix_first_cols(nc, pj1n, x1v)

    # GpSimd: q1 = t_j - 2*x1 at natural positions (parity crossing).
    # Pool has no TensorScalarPtr, so ACT predoubles x1 and GpSimd subtracts.
    nc.scalar.activation(x1d[:], x1, COPY, bias=0.0, scale=2.0)
    nc.gpsimd.tensor_sub(q1[:, 1:G - 1], tjq[:, 2:G], x1d[:, 1:G - 1])
    # GpSimd: V = A1c * P1c at natural positions (parity crossing)
    nc.gpsimd.tensor_mul(v[:], pj0[:, 1:1 + G], pj1[:, 1:1 + G])

    # ---- i-direction stencils (stacked, all even) ------------------------
    # pai = xc[+64] - xc[-64]: [A0c | P0c]; seam garbage = end rows, fixed.
    nc.vector.tensor_sub(pai[:, N:G2 - N], xc[:, 2 * N:G2], xc[:, 0:G2 - 2 * N])
    paiv = pai[:].rearrange("p (c h w) -> p c h w", c=2, h=N)
    _fix_first_rows(nc, paiv, xcv, 0)
    _fix_first_rows(nc, paiv, xcv, 1)

    # BC ch1: rows 0,63 of P0c scaled by -54.4 -> fp32 bc1, stored dense.
    nc.scalar.activation(bc1[:], paiv[:, 1, 0:N:N - 1, :], COPY,
                         bias=0.0, scale=-BC_SCALE)
    nc.sync.dma_start(out=out_ap[s0:s0 + P, 1, 0:N:N - 1, :], in_=bc1[:])
    # BC ch2: cols 0,63 of P1c scaled by +54.4 into the bf16 zero plane.
    bc2v = bc2[:].rearrange("p (h w) -> p h w", h=N)
    nc.scalar.activation(bc2v[:, :, 0:N:N - 1], pj1n[:, :, 0:N:N - 1], COPY,
                         bias=0.0, scale=BC_SCALE)
    nc.gpsimd.dma_start(
        out=out_ap[s0:s0 + P, 2].rearrange("s h w -> s (h w)"), in_=bc2[:])

    # U = (C4*A0c) * P0c  (after pai fixes)
    nc.vector.scalar_tensor_tensor(u[:], pai[:, 0:G], C4, pai[:, G:G2],
                                   ALU.mult, ALU.mult)

    # t_i and q0 (x1 channel only)
    nc.vector.tensor_add(tiq[:, N:G - N], x1[:, 2 * N:G], x1[:, 0:G - 2 * N])
    nc.vector.scalar_tensor_tensor(q0[:, N:G - N], x1[:, N:G - N], -2.0,
                                   tiq[:, N:G - N], ALU.mult, ALU.add)
    q0v = q0[:].rearrange("p (h w) -> p h w", h=N)
    _fix_second_rows(nc, q0v, x1v)
    # q1 col fixes (after the GpSimd q1 write)
    q1v = q1[:].rearrange("p (h w) -> p h w", h=N)
    _fix_second_cols(nc, q1v, x1v)

    # ---- combine (in-place over q0) --------------------------------------
    # S2 = q0 + q1 (corners come out right: one-sided in both directions)
    nc.vector.tensor_add(q0[:], q0[:], q1[:])
    # T = (x0 + 1) * S2
    nc.vector.scalar_tensor_tensor(q0[:], x0, 1.0, q0[:], ALU.add, ALU.mult)
    # r1 = -C*T - U
    nc.vector.scalar_tensor_tensor(q0[:], q0[:], -C, u[:], ALU.mult,
                                   ALU.subtract)
    # res = -C4*V + r1
    nc.vector.scalar_tensor_tensor(q0[:], v[:], -C4, q0[:], ALU.mult, ALU.add)

    # source-term corners: res[0:8,0:8] -= 10 ; res[56:64,56:64] += 10
    nc.scalar.activation(q0v[:, 0:8, 0:8], q0v[:, 0:8, 0:8], COPY,
                         bias=-10.0, scale=1.0)
    nc.scalar.activation(q0v[:, N - 8:N, N - 8:N], q0v[:, N - 8:N, N - 8:N],
                         COPY, bias=10.0, scale=1.0)
    # store: bf16 -> fp32 cast during DMA (SWDGE)
    nc.gpsimd.dma_start(
        out=out_ap[s0:s0 + P, 0].rearrange("s h w -> s (h w)"), in_=q0[:])


_WAITSPLIT_N = [0]


def _split_excess_waits(nc, max_waits=1):
    """Engine compute-instruction ISA structs hold only one sync-wait slot;
    Tile can assign several at cross-engine join points ("Too many sync wait
    commands" at codegen).  Move all but one wait onto InstNoOp carriers
    inserted just before, on the same engine."""
    keep = (mybir.InstEventSemaphore,
            mybir.InstCall, mybir.InstUnconditionalBranch, mybir.InstNoOp,
            mybir.InstRegisterMove, mybir.InstISA)
    for f in nc.m.functions:
        for b in f.blocks:
            new_insts = []
            for inst in b.instructions:
                si = inst.sync_info
                if (si is not None and si.on_wait and len(si.on_wait) > max_waits
                        and not isinstance(inst, keep)
                        and getattr(inst, "engine", None) is not None):
                    waits = list(si.on_wait)
                    excess, rest = waits[:-max_waits], waits[-max_waits:]
                    for w in excess:
                        _WAITSPLIT_N[0] += 1
                        nop = mybir.InstNoOp(
                            name=f"waitsplit_{_WAITSPLIT_N[0]}",
                            engine=inst.engine,
                            sync_info=mybir.SyncInfo(on_wait=[w], on_update=[]),
                            bass_nofuse=True,
                        )
                        new_insts.append(nop)
                    inst.sync_info = mybir.SyncInfo(on_wait=rest,
                                                    on_update=list(si.on_update))
                new_insts.append(inst)
            b.instructions = new_insts


def build_bass(split_waits=True):
    nc = bass.Bass()
    x = nc.declare_dram_parameter("x", [S_PER_CORE, 2, N, N], F32,
                                  isOutput=False)
    out = nc.declare_dram_parameter("out", [S_PER_CORE, 3, N, N], F32,
                                    isOutput=True)
    with tile.TileContext(nc) as tc:
        with ExitStack() as ctx:
            pool = ctx.enter_context(tc.tile_pool(name="scratch", bufs=1))
            n_tiles = S_PER_CORE // P

            # bf16 zero plane for BC ch2 (memset once; cols {0,63} rewritten
            # each tile; cast-stored to fp32 densely)
            bc2 = pool.tile([P, G], BF16, tag="bc2", name="bc2")
            nc.gpsimd.memset(bc2[:], 0.0)

            # shared across tiles (short lifetimes): tiq, u, x1d
            tiq = pool.tile([P, G], BF16, tag="tiq", name="tiq")
            u = pool.tile([P, G], BF16, tag="u", name="u")
            x1d = pool.tile([P, G], BF16, tag="x1d", name="x1d")

            sets = []
            for t in range(2):
                xc = pool.tile([P, G2], BF16, tag=f"xc{t}", name=f"xc{t}")
                pai = pool.tile([P, G2], BF16, tag=f"pai{t}", name=f"pai{t}")
                q0 = pool.tile([P, G], BF16, tag=f"q0_{t}", name=f"q0_{t}")
                pj0 = pool.tile([P, G + 2], BF16, tag=f"pj0_{t}",
                                name=f"pj0_{t}")
                pj1 = pool.tile([P, G + 2], BF16, tag=f"pj1_{t}",
                                name=f"pj1_{t}")
                tjq = pool.tile([P, G + 2], BF16, tag=f"tjq{t}", name=f"tjq{t}")
                q1 = pool.tile([P, G], BF16, tag=f"q1_{t}", name=f"q1_{t}")
                v = pool.tile([P, G], BF16, tag=f"v{t}", name=f"v{t}")
                bc1 = pool.tile([P, 2, N], F32, tag=f"bc1_{t}", name=f"bc1_{t}")
                sets.append((xc, pai, tiq, q0, u, pj0, pj1, tjq, q1, v, bc1))

            for it in range(n_tiles):
                _emit_tile(tc, x[:], out[:], it * P, sets[it % 2], bc2, x1d,
                           last_tile=(it == n_tiles - 1))
    if split_waits:
        _split_excess_waits(nc)
    return nc


_NC = None


def _get_nc():
    global _NC
    if _NC is None:
        _NC = build_bass()
    return _NC


def _axon_device_reset():
    """Recover a wedged NeuronCore (NRT_EXEC_UNIT_UNRECOVERABLE) via the
    axon client's reset entry point."""
    try:
        import ctypes

        import jax

        jax.devices()
        lib = ctypes.CDLL("/opt/axon/libaxon_pjrt.so")
        lib.axon_reset.restype = ctypes.c_int64
        return int(lib.axon_reset()) == 0
    except Exception:
        return False


def kernel(x0_pred, compute_bc=1, **_):
    from concourse.bass_utils import run_bass_kernel_spmd

    x = np.ascontiguousarray(np.asarray(x0_pred), dtype=np.float32)
    assert x.shape == (B, 2, N, N), x.shape
    nc = _get_nc()
    shards = x.reshape(N_CORES, S_PER_CORE, 2, N, N)
    in_maps = [{"x": shards[i]} for i in range(N_CORES)]
    try:
        res = run_bass_kernel_spmd(nc, in_maps, list(range(N_CORES)))
    except Exception:
        if not _axon_device_reset():
            raise
        res = run_bass_kernel_spmd(nc, in_maps, list(range(N_CORES)))
    full = np.concatenate([res.results[i]["out"] for i in range(N_CORES)],
                          axis=0)
    if not int(np.asarray(compute_bc)):
        return full[:, :1]
    return full


# revision 7
# speedup vs baseline: 1.3504x; 1.1941x over previous
"""Trainium2 Bass kernel for the Darcy64 residual (dense stencil + BC extraction).

Contract: kernel(**inputs) takes the FULL inputs from setup_inputs()
(x0_pred [2048,2,64,64] f32, compute_bc scalar) and returns the FULL
output [2048,3,64,64] f32 (or [2048,1,64,64] if compute_bc is falsy).

Strategy: pure data parallel over 8 NeuronCores (256 samples each),
128 samples per tile on SBUF partitions, each sample's [64,64] grid
flattened along the free dim.  v3 design notes:

  * All compute in bf16 so DVE tensor_tensor runs in the 2x_1p perf mode.
    scalar_tensor_tensor has NO 2x uop (measured 1x on HW), so every
    interior op must be a plain tensor_tensor: the scalar constants are
    folded away by prescaling x1 by -C/4 on the ACT engine (the residual
    is linear in x1, so the scale propagates through every x1-derived
    term), plus ACT-precomputed 2*x1' and 4*(x0+1) operand tensors.
      res = 4(x0+1)*S2' + A0*P0' + A1*P1' - f_s   (primes: from -C/4*x1)
  * j-direction stencils write +1-shifted buffers (buf[k] = val[k-1]) so
    every big DVE op keeps 4-byte-aligned APs (2x mode requires that).
    The two parity-crossing merges (q1 = t_j - 2x1', V = A1*P1') run on
    GpSimd, which has no alignment-gated perf modes.
  * Inputs are cast fp32->bf16 during the DMA load (SWDGE, prefetched
    up front); the residual and the dense-zero BC ch2 plane are cast
    bf16->fp32 during the store.

Boundary handling (d = 1/64, flat index = i*64 + j):
  First-derivative end rows/cols use the *first*-end coefficients at both
  ends, which flips the sign of the last row/col; the flip cancels in the
  products A*P and makes the BC extraction scale uniform.
  f_s is zero except +10 on grid [0:8,0:8] and -10 on [56:64,56:64].
  BC ch1 rows = (BC_SCALE/C4)*P0' rows; ch2 cols = -(BC_SCALE/C4)*P1'.
  ch1 relies on the runtime's pre-zeroed output buffers (only rows 0,63
  stored); ch2 stores a dense bf16 zero plane with the two columns set.
"""

import sys
from contextlib import ExitStack

import numpy as np

sys.path.insert(0, "/opt/trn_rl_repo")

import concourse.bass as bass  # noqa: E402
import concourse.tile as tile  # noqa: E402
from concourse import mybir  # noqa: E402

N_CORES = 8
B = 2048
S_PER_CORE = B // N_CORES  # 256
P = 128                    # samples per tile (partition dim)
N = 64
G = N * N                  # 4096
C = 39.1 * float(N * N)    # 39.1 / d^2 = 160153.6
C4 = C / 4.0
BC_SCALE = 1.7 * (N / 2.0)  # 1.7/(2d) = 54.4
BC1_SCALE = BC_SCALE / C4   # ch1 scale on the prescaled P0'
BC2_SCALE = -BC_SCALE / C4  # ch2 scale on the prescaled P1'

F32 = mybir.dt.float32
BF16 = mybir.dt.bfloat16
ALU = mybir.AluOpType
COPY = mybir.ActivationFunctionType.Copy


def _fix_first_rows(nc, dstv, srcv):
    """First-derivative one-sided ends on rows 0 and 63 ([p, 64, 64] views).
    First-end coefficients mirrored at the far end (sign flip there cancels
    in products / makes the BC scale uniform)."""
    for (r0, r1, r2) in ((0, 1, 2), (N - 1, N - 2, N - 3)):
        d = dstv[:, r0:r0 + 1, :]
        f0 = srcv[:, r0:r0 + 1, :]
        f1 = srcv[:, r1:r1 + 1, :]
        f2 = srcv[:, r2:r2 + 1, :]
        nc.vector.scalar_tensor_tensor(d, f0, -3.0, f2, ALU.mult, ALU.subtract)
        nc.vector.scalar_tensor_tensor(d, f1, 4.0, d, ALU.mult, ALU.add)


def _fix_second_rows(nc, dstv, srcv):
    for (r0, r1, r2, r3) in ((0, 1, 2, 3), (N - 1, N - 2, N - 3, N - 4)):
        d = dstv[:, r0:r0 + 1, :]
        f0 = srcv[:, r0:r0 + 1, :]
        f1 = srcv[:, r1:r1 + 1, :]
        f2 = srcv[:, r2:r2 + 1, :]
        f3 = srcv[:, r3:r3 + 1, :]
        nc.vector.scalar_tensor_tensor(d, f0, 2.0, f3, ALU.mult, ALU.subtract)
        nc.vector.scalar_tensor_tensor(d, f1, -5.0, d, ALU.mult, ALU.add)
        nc.vector.scalar_tensor_tensor(d, f2, 4.0, d, ALU.mult, ALU.add)


def _fix_first_cols(nc, dstv, srcv):
    for (c0, c1, c2) in ((0, 1, 2), (N - 1, N - 2, N - 3)):
        d = dstv[:, :, c0:c0 + 1]
        f0 = srcv[:, :, c0:c0 + 1]
        f1 = srcv[:, :, c1:c1 + 1]
        f2 = srcv[:, :, c2:c2 + 1]
        nc.vector.scalar_tensor_tensor(d, f0, -3.0, f2, ALU.mult, ALU.subtract)
        nc.vector.scalar_tensor_tensor(d, f1, 4.0, d, ALU.mult, ALU.add)


def _fix_second_cols(nc, dstv, srcv):
    for (c0, c1, c2, c3) in ((0, 1, 2, 3), (N - 1, N - 2, N - 3, N - 4)):
        d = dstv[:, :, c0:c0 + 1]
        f0 = srcv[:, :, c0:c0 + 1]
        f1 = srcv[:, :, c1:c1 + 1]
        f2 = srcv[:, :, c2:c2 + 1]
        f3 = srcv[:, :, c3:c3 + 1]
        nc.vector.scalar_tensor_tensor(d, f0, 2.0, f3, ALU.mult, ALU.subtract)
        nc.vector.scalar_tensor_tensor(d, f1, -5.0, d, ALU.mult, ALU.add)
        nc.vector.scalar_tensor_tensor(d, f2, 4.0, d, ALU.mult, ALU.add)


def _emit_tile(tc, out_ap, s0, bufs, shared, last_tile):
    """Emit one 128-sample tile starting at sample s0 (within this core).
    x1s holds -C4*x1 (prescaled by ACT, in place over the loaded x1)."""
    nc = tc.nc
    x0b, x1s, a0c, p0c, q0, pj0, pj1, q1, v, bc1 = bufs
    x1d, tiq, w4, u, tjq, bc2 = shared

    x0v = x0b[:].rearrange("p (h w) -> p h w", h=N)

    # ---- ACT prescales -------------------------------------------------
    # x1' = -C4 * x1 (in place); x1d = 2*x1'; w4 = 4*(x0+1)
    nc.scalar.activation(x1s[:], x1s[:], COPY, bias=0.0, scale=-C4)
    x1v = x1s[:].rearrange("p (h w) -> p h w", h=N)
    nc.scalar.activation(x1d[:], x1s[:], COPY, bias=0.0, scale=2.0)
    nc.scalar.activation(w4[:], x0b[:], COPY, bias=4.0, scale=4.0)

    # ---- j-direction stencils into +1-shifted buffers (APs all even) ----
    # tjq[k] = x1'[k] + x1'[k-2]  (t_j shifted +1); pj1[k] = P1'[k-1]
    nc.vector.tensor_add(tjq[:, 2:G], x1s[:, 2:G], x1s[:, 0:G - 2])
    nc.vector.tensor_sub(pj1[:, 2:G], x1s[:, 2:G], x1s[:, 0:G - 2])
    pj1n = pj1[:, 1:1 + G].rearrange("p (h w) -> p h w", h=N)
    _fix_first_cols(nc, pj1n, x1v)
    # GpSimd: q1 = t_j - 2*x1' at natural positions (parity crossing)
    nc.gpsimd.tensor_sub(q1[:, 1:G - 1], tjq[:, 2:G], x1d[:, 1:G - 1])

    nc.vector.tensor_sub(pj0[:, 2:G], x0b[:, 2:G], x0b[:, 0:G - 2])
    pj0n = pj0[:, 1:1 + G].rearrange("p (h w) -> p h w", h=N)
    _fix_first_cols(nc, pj0n, x0v)
    # GpSimd: V = A1 * P1' at natural positions (parity crossing)
    nc.gpsimd.tensor_mul(v[:], pj0[:, 1:1 + G], pj1[:, 1:1 + G])

    # ---- i-direction stencils ------------------------------------------
    nc.vector.tensor_sub(p0c[:, N:G - N], x1s[:, 2 * N:G], x1s[:, 0:G - 2 * N])
    p0v = p0c[:].rearrange("p (h w) -> p h w", h=N)
    _fix_first_rows(nc, p0v, x1v)
    nc.vector.tensor_sub(a0c[:, N:G - N], x0b[:, 2 * N:G], x0b[:, 0:G - 2 * N])
    a0v = a0c[:].rearrange("p (h w) -> p h w", h=N)
    _fix_first_rows(nc, a0v, x0v)

    # BC ch1: rows 0,63 of P0' -> fp32 bc1, stored dense (2 rows/sample).
    nc.scalar.activation(bc1[:], p0v[:, 0:N:N - 1, :], COPY,
                         bias=0.0, scale=BC1_SCALE)
    nc.sync.dma_start(out=out_ap[s0:s0 + P, 1, 0:N:N - 1, :], in_=bc1[:])
    # BC ch2: cols 0,63 of P1' into the bf16 zero plane, dense cast-store.
    bc2v = bc2[:].rearrange("p (h w) -> p h w", h=N)
    nc.scalar.activation(bc2v[:, :, 0:N:N - 1], pj1n[:, :, 0:N:N - 1], COPY,
                         bias=0.0, scale=BC2_SCALE)
    nc.gpsimd.dma_start(
        out=out_ap[s0:s0 + P, 2].rearrange("s h w -> s (h w)"), in_=bc2[:])

    # U = A0 * P0'
    nc.vector.tensor_mul(u[:], a0c[:], p0c[:])

    # t_i and q0 (x1' channel)
    nc.vector.tensor_add(tiq[:, N:G - N], x1s[:, 2 * N:G], x1s[:, 0:G - 2 * N])
    nc.vector.tensor_sub(q0[:, N:G - N], tiq[:, N:G - N], x1d[:, N:G - N])
    q0v = q0[:].rearrange("p (h w) -> p h w", h=N)
    _fix_second_rows(nc, q0v, x1v)
    q1v = q1[:].rearrange("p (h w) -> p h w", h=N)
    _fix_second_cols(nc, q1v, x1v)

    # ---- combine (in-place over q0) ------------------------------------
    nc.vector.tensor_add(q0[:], q0[:], q1[:])   # S2'
    nc.vector.tensor_mul(q0[:], w4[:], q0[:])   # 4(x0+1)*S2'
    nc.vector.tensor_add(q0[:], q0[:], u[:])    # + A0*P0'
    nc.vector.tensor_add(q0[:], q0[:], v[:])    # + A1*P1'  (= res before f_s)

    # source-term corners: res[0:8,0:8] -= 10 ; res[56:64,56:64] += 10
    nc.scalar.activation(q0v[:, 0:8, 0:8], q0v[:, 0:8, 0:8], COPY,
                         bias=-10.0, scale=1.0)
    nc.scalar.activation(q0v[:, N - 8:N, N - 8:N], q0v[:, N - 8:N, N - 8:N],
                         COPY, bias=10.0, scale=1.0)
    # store: bf16 -> fp32 cast during DMA (SWDGE)
    nc.gpsimd.dma_start(
        out=out_ap[s0:s0 + P, 0].rearrange("s h w -> s (h w)"), in_=q0[:])


_WAITSPLIT_N = [0]


def _split_excess_waits(nc, max_waits=1):
    """Engine compute-instruction ISA structs hold only one sync-wait slot;
    Tile can assign several at cross-engine join points ("Too many sync wait
    commands" at codegen).  Move all but one wait onto InstNoOp carriers
    inserted just before, on the same engine."""
    keep = (mybir.InstEventSemaphore,
            mybir.InstCall, mybir.InstUnconditionalBranch, mybir.InstNoOp,
            mybir.InstRegisterMove, mybir.InstISA)
    for f in nc.m.functions:
        for b in f.blocks:
            new_insts = []
            for inst in b.instructions:
                si = inst.sync_info
                if (si is not None and si.on_wait and len(si.on_wait) > max_waits
                        and not isinstance(inst, keep)
                        and getattr(inst, "engine", None) is not None):
                    waits = list(si.on_wait)
                    excess, rest = waits[:-max_waits], waits[-max_waits:]
                    for w in excess:
                        _WAITSPLIT_N[0] += 1
                        nop = mybir.InstNoOp(
                            name=f"waitsplit_{_WAITSPLIT_N[0]}",
                            engine=inst.engine,
                            sync_info=mybir.SyncInfo(on_wait=[w], on_update=[]),
                            bass_nofuse=True,
                        )
                        new_insts.append(nop)
                    inst.sync_info = mybir.SyncInfo(on_wait=rest,
                                                    on_update=list(si.on_update))
                new_insts.append(inst)
            b.instructions = new_insts


def build_bass(split_waits=True):
    nc = bass.Bass()
    x = nc.declare_dram_parameter("x", [S_PER_CORE, 2, N, N], F32,
                                  isOutput=False)
    out = nc.declare_dram_parameter("out", [S_PER_CORE, 3, N, N], F32,
                                    isOutput=True)
    with tile.TileContext(nc) as tc:
        with ExitStack() as ctx:
            pool = ctx.enter_context(tc.tile_pool(name="scratch", bufs=1))
            n_tiles = S_PER_CORE // P

            # shared across tiles (short lifetimes / serialization is cheap)
            x1d = pool.tile([P, G], BF16, tag="x1d", name="x1d")
            tiq = pool.tile([P, G], BF16, tag="tiq", name="tiq")
            w4 = pool.tile([P, G], BF16, tag="w4", name="w4")
            u = pool.tile([P, G], BF16, tag="u", name="u")
            tjq = pool.tile([P, G + 2], BF16, tag="tjq", name="tjq")
            bc2 = pool.tile([P, G], BF16, tag="bc2", name="bc2")
            nc.gpsimd.memset(bc2[:], 0.0)
            shared = (x1d, tiq, w4, u, tjq, bc2)

            sets = []
            for t in range(2):
                x0b = pool.tile([P, G], BF16, tag=f"x0b{t}", name=f"x0b{t}")
                x1s = pool.tile([P, G], BF16, tag=f"x1s{t}", name=f"x1s{t}")
                a0c = pool.tile([P, G], BF16, tag=f"a0c{t}", name=f"a0c{t}")
                p0c = pool.tile([P, G], BF16, tag=f"p0c{t}", name=f"p0c{t}")
                q0 = pool.tile([P, G], BF16, tag=f"q0_{t}", name=f"q0_{t}")
                pj0 = pool.tile([P, G + 2], BF16, tag=f"pj0_{t}",
                                name=f"pj0_{t}")
                pj1 = pool.tile([P, G + 2], BF16, tag=f"pj1_{t}",
                                name=f"pj1_{t}")
                q1 = pool.tile([P, G], BF16, tag=f"q1_{t}", name=f"q1_{t}")
                v = pool.tile([P, G], BF16, tag=f"v{t}", name=f"v{t}")
                bc1 = pool.tile([P, 2, N], F32, tag=f"bc1_{t}", name=f"bc1_{t}")
                sets.append((x0b, x1s, a0c, p0c, q0, pj0, pj1, q1, v, bc1))

            # prefetch all loads up front (x1 halves first: ACT prescale and
            # the whole DVE chain consume x1); fp32 -> bf16 SWDGE cast
            for it in range(n_tiles):
                nc.gpsimd.dma_start(
                    out=sets[it % 2][1][:],
                    in_=x[:][it * P:(it + 1) * P, 1].rearrange(
                        "s h w -> s (h w)"))
            for it in range(n_tiles):
                nc.gpsimd.dma_start(
                    out=sets[it % 2][0][:],
                    in_=x[:][it * P:(it + 1) * P, 0].rearrange(
                        "s h w -> s (h w)"))

            for it in range(n_tiles):
                _emit_tile(tc, out[:], it * P, sets[it % 2], shared,
                           last_tile=(it == n_tiles - 1))
    if split_waits:
        _split_excess_waits(nc)
    return nc


_NC = None


def _get_nc():
    global _NC
    if _NC is None:
        _NC = build_bass()
    return _NC


def _axon_device_reset():
    """Recover a wedged NeuronCore (NRT_EXEC_UNIT_UNRECOVERABLE) via the
    axon client's reset entry point."""
    try:
        import ctypes

        import jax

        jax.devices()
        lib = ctypes.CDLL("/opt/axon/libaxon_pjrt.so")
        lib.axon_reset.restype = ctypes.c_int64
        return int(lib.axon_reset()) == 0
    except Exception:
        return False


def kernel(x0_pred, compute_bc=1, **_):
    from concourse.bass_utils import run_bass_kernel_spmd

    x = np.ascontiguousarray(np.asarray(x0_pred), dtype=np.float32)
    assert x.shape == (B, 2, N, N), x.shape
    nc = _get_nc()
    shards = x.reshape(N_CORES, S_PER_CORE, 2, N, N)
    in_maps = [{"x": shards[i]} for i in range(N_CORES)]
    try:
        res = run_bass_kernel_spmd(nc, in_maps, list(range(N_CORES)))
    except Exception:
        if not _axon_device_reset():
            raise
        res = run_bass_kernel_spmd(nc, in_maps, list(range(N_CORES)))
    full = np.concatenate([res.results[i]["out"] for i in range(N_CORES)],
                          axis=0)
    if not int(np.asarray(compute_bc)):
        return full[:, :1]
    return full
